# revision 1
# baseline (speedup 1.0000x reference)
"""LlamaAttention (B=1, S=2048, H=4096, 32 heads / 8 KV heads) on 8 TRN2 NeuronCores.

Sharding: tensor-parallel over heads. Core c owns Q heads [4c, 4c+4) and KV head c
(Wq/Wk/Wv column shards, Wo row shard). Each core computes a full [S, H] partial
output; the host sums the 8 partials (the all-reduce for row-sharded Wo).

Per-core dataflow (all matmuls fp32r = full-rate fp32), interleaved per sq-block:
  - projections: QT/KT/VT computed *transposed* (lhsT = W k-tiles, rhs = XT k-tiles,
    both streamed from HBM; Wk/Wv resident), RoPE applied straight out of PSUM,
    VT transposed to V-natural via PE transposes.
  - attention per (head, sq-block): PT = exp(scale * KT.T @ QT) over causal sk tiles
    (diagonal tiles masked by precomputed 0/1 masks), outT accumulated as V.T @ PT
    in PSUM, softmax sums accumulated on DVE, partition-reduced on GPSIMD,
    normalization fused into the PSUM->SBUF epilogue producing attnT.
  - output projection out = attnT.T @ Wo: previous block's chunks are issued between
    attention heads as PE filler while softmax tails (reduce/normalize) drain.
"""

import numpy as np

HIDDEN = 4096
N_HEADS = 32
N_KV = 8
HD = 128
S = 2048
N_CORES = 8
HPC = N_HEADS // N_CORES          # 4 Q heads per core
DQ = HPC * HD                     # 512 q columns per core
ROPE_BASE = 10000.0
SCALE = 1.0 / float(np.sqrt(HD))

NBLK = S // 512                   # 4 sq blocks of 512
NSK = S // 128                    # 16 sk tiles of 128
KT_TILES = HIDDEN // 128          # 32 contraction tiles

_CACHE = {}


def _build():
    import concourse.bass as bass
    import concourse.tile as tile
    from concourse import bacc, mybir
    import concourse.bass_isa as bass_isa

    f32 = mybir.dt.float32
    f32r = mybir.dt.float32r
    EXP = mybir.ActivationFunctionType.Exp
    CPY = mybir.ActivationFunctionType.Copy
    ADD = bass_isa.ReduceOp.add

    nc = bacc.Bacc("TRN2", target_bir_lowering=False, debug=False,
                   num_devices=N_CORES)

    xt_d = nc.dram_tensor("xt", [HIDDEN, S], f32r, kind="ExternalInput").ap()
    wq_d = nc.dram_tensor("wq", [HIDDEN, DQ], f32r, kind="ExternalInput").ap()
    wk_d = nc.dram_tensor("wk", [HIDDEN, HD], f32r, kind="ExternalInput").ap()
    wv_d = nc.dram_tensor("wv", [HIDDEN, HD], f32r, kind="ExternalInput").ap()
    wo_d = nc.dram_tensor("wo", [DQ, HIDDEN], f32r, kind="ExternalInput").ap()
    cos_d = nc.dram_tensor("cosT", [HD, S], f32, kind="ExternalInput").ap()
    sin_d = nc.dram_tensor("sinS", [HD, S], f32, kind="ExternalInput").ap()
    msk_d = nc.dram_tensor("masks", [128, 896], f32r, kind="ExternalInput").ap()
    idn_d = nc.dram_tensor("ident", [128, 128], f32r, kind="ExternalInput").ap()
    out_d = nc.dram_tensor("out", [S, HIDDEN], f32, kind="ExternalOutput").ap()

    with tile.TileContext(nc) as tc:
        from contextlib import ExitStack
        with ExitStack() as ctx:
            ep = ctx.enter_context
            consts = ep(tc.tile_pool(name="consts", bufs=1))
            main = ep(tc.tile_pool(name="main", bufs=1))
            wqs_pool = ep(tc.tile_pool(name="wqs", bufs=2))
            xt_pool = ep(tc.tile_pool(name="xtp", bufs=3))
            tmp_pool = ep(tc.tile_pool(name="tmpp", bufs=2))
            et_pool = ep(tc.tile_pool(name="etp", bufs=3))
            sums_pool = ep(tc.tile_pool(name="sumsp", bufs=2))
            sbc_pool = ep(tc.tile_pool(name="sbcp", bufs=2))
            vt_pool = ep(tc.tile_pool(name="vtsbp", bufs=2))
            osb_pool = ep(tc.tile_pool(name="osbp", bufs=2))
            wos_pool = ep(tc.tile_pool(name="wosp", bufs=2))
            ps_a = ep(tc.tile_pool(name="psa", bufs=4, space="PSUM"))
            ps_k = ep(tc.tile_pool(name="psk", bufs=1, space="PSUM"))
            ps_pt = ep(tc.tile_pool(name="pspt", bufs=1, space="PSUM"))
            ps_vt = ep(tc.tile_pool(name="psvt", bufs=1, space="PSUM"))
            ps_ot = ep(tc.tile_pool(name="psot", bufs=1, space="PSUM"))

            # constants / resident weights (chunked so first consumers start early)
            ident = consts.tile([128, 128], f32r)
            nc.sync.dma_start(out=ident, in_=idn_d)
            masks = consts.tile([128, 896], f32r)
            nc.sync.dma_start(out=masks, in_=msk_d)
            cosT = consts.tile([HD, S], f32)
            sinS = consts.tile([HD, S], f32)
            wk = consts.tile([128, KT_TILES, HD], f32r)
            wv = consts.tile([128, KT_TILES, HD], f32r)
            wk_r = wk_d.rearrange("(kt p) m -> p kt m", p=128)
            wv_r = wv_d.rearrange("(kt p) m -> p kt m", p=128)
            for q in range(4):
                ksl = slice(q * 8, (q + 1) * 8)
                nc.sync.dma_start(out=wk[:, ksl, :], in_=wk_r[:, ksl, :])
                nc.sync.dma_start(out=wv[:, ksl, :], in_=wv_r[:, ksl, :])
            nc.sync.dma_start(out=cosT, in_=cos_d)
            nc.sync.dma_start(out=sinS, in_=sin_d)
            wo_r = wo_d.rearrange("(hh p) m -> p hh m", p=128)

            # long-lived activations
            qt = [main.tile([128, S], f32r, tag=f"qt{h}", name=f"qt{h}")
                  for h in range(HPC)]
            kt = main.tile([128, S], f32r)
            v_sb = main.tile([128, NSK, 128], f32r)
            at = [main.tile([128, S], f32r, tag=f"at{h}", name=f"at{h}")
                  for h in range(HPC)]

            def rope(ps, dst, blk):
                """dst[:, blk*512:+512] = rope(ps) using cosT/sinS tables."""
                lo = blk * 512
                sl = slice(lo, lo + 512)
                t = tmp_pool.tile([128, 512], f32, tag="ropetmp", name="ropetmp")
                nc.vector.tensor_mul(t[0:64, :], ps[64:128, :], sinS[0:64, sl])
                nc.vector.tensor_mul(t[64:128, :], ps[0:64, :], sinS[64:128, sl])
                nc.vector.tensor_mul(dst[:, sl], ps[:], cosT[:, sl])
                nc.vector.tensor_add(dst[:, sl], dst[:, sl], t[:])

            def proj_block(blk):
                """QT/KT/V for sq block `blk`; leaves q psums + rope hooks."""
                lo = blk * 512
                q_ps = [ps_a.tile([128, 512], f32, tag="psA", name=f"qps{h}")
                        for h in range(HPC)]
                k_ps = ps_k.tile([128, 512], f32, tag="psK", name="kps")
                vt_ps = ps_vt.tile([128, 512], f32, tag="psVT", name="vtps")
                for k in range(KT_TILES):
                    wq_t = wqs_pool.tile([128, DQ], f32r, tag="wqt", name="wqt")
                    nc.sync.dma_start(out=wq_t,
                                      in_=wq_d[k * 128:(k + 1) * 128, :])
                    x_t = xt_pool.tile([128, 512], f32r, tag="xt", name="xt")
                    nc.sync.dma_start(out=x_t,
                                      in_=xt_d[k * 128:(k + 1) * 128, lo:lo + 512])
                    st = (k == 0)
                    sp = (k == KT_TILES - 1)
                    for h in range(HPC):
                        nc.tensor.matmul(q_ps[h][:], wq_t[:, h * 128:(h + 1) * 128],
                                         x_t[:], start=st, stop=sp)
                    nc.tensor.matmul(k_ps[:], wk[:, k, :], x_t[:], start=st, stop=sp)
                    nc.tensor.matmul(vt_ps[:], wv[:, k, :], x_t[:], start=st, stop=sp)

                rope(k_ps, kt, blk)
                # V: VT psum -> SBUF, then 4 PE transposes to V natural
                vt_sb = vt_pool.tile([128, 512], f32r, tag="vtsb", name="vtsb")
                nc.vector.tensor_copy(vt_sb[:], vt_ps[:])
                for t in range(4):
                    vp = ps_vt.tile([128, 128], f32r, tag="psVT", name="vtr")
                    nc.tensor.transpose(vp[:], vt_sb[:, t * 128:(t + 1) * 128],
                                        ident[:])
                    nc.scalar.activation(v_sb[:, blk * 4 + t, :], vp[:], CPY)
                return q_ps

            def attn_head(h, blk):
                lo = blk * 512
                nsk = 4 * (blk + 1)
                ot_ps = ps_ot.tile([128, 512], f32, tag="psOT", name="otps")
                sums = sums_pool.tile([128, 512], f32, tag="sums", name="sums")
                for i in range(nsk):
                    pt = ps_pt.tile([128, 512], f32, tag="psPT", name="pt")
                    nc.tensor.matmul(pt[:], kt[:, i * 128:(i + 1) * 128],
                                     qt[h][:, lo:lo + 512], start=True, stop=True)
                    et = et_pool.tile([128, 512], f32r, tag="et", name="et")
                    nc.scalar.activation(et[:], pt[:], EXP, scale=SCALE)
                    off = i - 4 * blk
                    if off >= 0:
                        w0 = (3 - off) * 128
                        nc.vector.tensor_mul(
                            et[:], et[:], masks[:, w0:w0 + 512])
                    if i == 0:
                        nc.vector.tensor_copy(sums[:], et[:])
                    else:
                        nc.vector.tensor_add(sums[:], sums[:], et[:])
                    nc.tensor.matmul(ot_ps[:], v_sb[:, i, :], et[:],
                                     start=(i == 0), stop=(i == nsk - 1))
                sbc = sbc_pool.tile([128, 512], f32, tag="sbc", name="sbc")
                nc.gpsimd.partition_all_reduce(sbc[:], sums[:], channels=128,
                                               reduce_op=ADD)
                nc.vector.reciprocal(sbc[:], sbc[:])
                nc.vector.tensor_mul(at[h][:, lo:lo + 512], ot_ps[:], sbc[:])

            # -------- interleaved schedule over sq blocks --------
            for blk in range(NBLK):
                q_ps = proj_block(blk)
                for h in range(HPC):
                    rope(q_ps[h], qt[h], blk)
                    attn_head(h, blk)

            # ---- phase 3 tail: out = attnT.T @ Wo, Wo streamed per n ----
            for n in range(HIDDEN // 512):
                wo_n = wos_pool.tile([128, HPC, 512], f32r, tag="won", name="won")
                nc.sync.dma_start(out=wo_n,
                                  in_=wo_r[:, :, n * 512:(n + 1) * 512])
                for m in range(NSK):
                    o_ps = ps_a.tile([128, 512], f32, tag="psA", name="ops")
                    for h in range(HPC):
                        nc.tensor.matmul(o_ps[:], at[h][:, m * 128:(m + 1) * 128],
                                         wo_n[:, h, :],
                                         start=(h == 0), stop=(h == HPC - 1))
                    o_sb = osb_pool.tile([128, 512], f32, tag="osb", name="osb")
                    if m % 2 == 0:
                        nc.scalar.activation(o_sb[:], o_ps[:], CPY)
                    else:
                        nc.vector.tensor_copy(o_sb[:], o_ps[:])
                    nc.sync.dma_start(
                        out=out_d[m * 128:(m + 1) * 128, n * 512:(n + 1) * 512],
                        in_=o_sb[:])

    nc.compile()
    return nc


def _host_prep(hidden_states, position_ids, Wq, Wk, Wv, Wo):
    X = np.ascontiguousarray(
        np.asarray(hidden_states, dtype=np.float32).reshape(S, HIDDEN))
    XT = np.ascontiguousarray(X.T)

    pos = np.asarray(position_ids).reshape(-1)[:S].astype(np.float32)
    inv = (1.0 / (ROPE_BASE ** (np.arange(0, HD, 2, dtype=np.float32) / HD))
           ).astype(np.float32)
    freqs = pos[:, None] * inv[None, :]              # [S, 64]
    cos_h = np.cos(freqs).astype(np.float32)         # [S, 64] (= both halves)
    sin_h = np.sin(freqs).astype(np.float32)
    cosT = np.ascontiguousarray(np.concatenate([cos_h, cos_h], axis=1).T)
    sinT = np.concatenate([sin_h, sin_h], axis=1).T
    sinS = np.ascontiguousarray(np.concatenate([-sinT[0:64], sinT[64:128]], axis=0))

    # sliding-window mask [zeros(3x128) | tri | ones(3x128)]: offset o slice
    # starts at (3-o)*128 and covers 512 cols -> c<o zero, c==o tri, c>o ones
    tri = (np.arange(128)[:, None] <= np.arange(128)[None, :]).astype(np.float32)
    masks = np.concatenate([np.zeros((128, 384), np.float32), tri,
                            np.ones((128, 384), np.float32)], axis=1)

    ident = np.eye(128, dtype=np.float32)

    Wq = np.asarray(Wq, dtype=np.float32)
    Wk = np.asarray(Wk, dtype=np.float32)
    Wv = np.asarray(Wv, dtype=np.float32)
    Wo = np.asarray(Wo, dtype=np.float32)

    in_maps = []
    for c in range(N_CORES):
        in_maps.append({
            "xt": XT,
            "wq": np.ascontiguousarray(Wq[:, c * DQ:(c + 1) * DQ]),
            "wk": np.ascontiguousarray(Wk[:, c * HD:(c + 1) * HD]),
            "wv": np.ascontiguousarray(Wv[:, c * HD:(c + 1) * HD]),
            "wo": np.ascontiguousarray(Wo[c * DQ:(c + 1) * DQ, :]),
            "cosT": cosT,
            "sinS": sinS,
            "masks": masks,
            "ident": ident,
        })
    return in_maps


def kernel(hidden_states, position_ids, Wq, Wk, Wv, Wo, _run_opts=None):
    from concourse.bass_utils import run_bass_kernel_spmd

    if "nc" not in _CACHE:
        _CACHE["nc"] = _build()
    nc = _CACHE["nc"]

    in_maps = _host_prep(hidden_states, position_ids, Wq, Wk, Wv, Wo)
    opts = dict(_run_opts or {})
    res = run_bass_kernel_spmd(nc, in_maps, core_ids=list(range(N_CORES)), **opts)
    _CACHE["last_result"] = res

    out = res.results[0]["out"].astype(np.float64)
    for c in range(1, N_CORES):
        out += res.results[c]["out"]
    return out.astype(np.float32).reshape(1, S, HIDDEN)



# revision 2
# speedup vs baseline: 1.9012x; 1.9012x over previous
"""LlamaAttention (B=1, S=2048, H=4096, 32 heads / 8 KV heads) on 8 TRN2 NeuronCores.

Sharding: tensor-parallel over heads. Core c owns Q heads [4c, 4c+4) and KV head c
(Wq/Wk/Wv column shards, Wo row shard). Each core computes a full [S, H] partial
output in bf16; the host sums the 8 partials (the all-reduce for row-sharded Wo).

v2 vs baseline: all matmul operands bf16 (half DMA/SBUF traffic, same PE rate),
all weights SBUF-resident (Wq was re-streamed 4x), xt block-resident with
prefetch, per-head projection passes to cut concurrent PSUM banks 6->4, PSUM
rebalanced so attention double-buffers PT, softmax tail shortened
(reciprocal_approx_fast instead of 3.3us DVE reciprocal) and hidden behind
out-projection filler matmuls interleaved into the next head's attention loop
(keeps the PE warm - the baseline re-throttled to 1.2 GHz at every head tail).
"""

import numpy as np

HIDDEN = 4096
N_HEADS = 32
N_KV = 8
HD = 128
S = 2048
N_CORES = 8
HPC = N_HEADS // N_CORES          # 4 Q heads per core
DQ = HPC * HD                     # 512 q columns per core
ROPE_BASE = 10000.0
SCALE = 1.0 / float(np.sqrt(HD))

NBLK = S // 512                   # 4 sq blocks of 512
NSK = S // 128                    # 16 sk tiles of 128
KT = HIDDEN // 128                # 32 contraction tiles

_CACHE = {}


def _build():
    import concourse.bass as bass
    import concourse.tile as tile
    from concourse import bacc, mybir
    import concourse.bass_isa as bass_isa

    f32 = mybir.dt.float32
    bf = mybir.dt.bfloat16
    EXP = mybir.ActivationFunctionType.Exp
    CPY = mybir.ActivationFunctionType.Copy
    ADD = bass_isa.ReduceOp.add

    nc = bacc.Bacc("TRN2", target_bir_lowering=False, debug=False,
                   num_devices=N_CORES)

    xt_d = nc.dram_tensor("xt", [HIDDEN, S], bf, kind="ExternalInput").ap()
    wq_d = nc.dram_tensor("wq", [HIDDEN, DQ], bf, kind="ExternalInput").ap()
    wk_d = nc.dram_tensor("wk", [HIDDEN, HD], bf, kind="ExternalInput").ap()
    wv_d = nc.dram_tensor("wv", [HIDDEN, HD], bf, kind="ExternalInput").ap()
    wo_d = nc.dram_tensor("wo", [DQ, HIDDEN], bf, kind="ExternalInput").ap()
    cos_d = nc.dram_tensor("cosT", [HD, S], f32, kind="ExternalInput").ap()
    sin_d = nc.dram_tensor("sinS", [HD, S], f32, kind="ExternalInput").ap()
    msk_d = nc.dram_tensor("masks", [128, 896], bf, kind="ExternalInput").ap()
    idn_d = nc.dram_tensor("ident", [128, 128], bf, kind="ExternalInput").ap()
    out_d = nc.dram_tensor("out", [S, HIDDEN], bf, kind="ExternalOutput").ap()

    with tile.TileContext(nc) as tc:
        from contextlib import ExitStack
        with ExitStack() as ctx:
            ep = ctx.enter_context
            consts = ep(tc.tile_pool(name="consts", bufs=1))
            main = ep(tc.tile_pool(name="main", bufs=1))
            tmp_pool = ep(tc.tile_pool(name="tmpp", bufs=2))
            et_pool = ep(tc.tile_pool(name="etp", bufs=6))
            sums_pool = ep(tc.tile_pool(name="sumsp", bufs=2))
            sbc_pool = ep(tc.tile_pool(name="sbcp", bufs=2))
            vt_pool = ep(tc.tile_pool(name="vtp", bufs=2))
            osb_pool = ep(tc.tile_pool(name="osbp", bufs=3))
            ps_kv = ep(tc.tile_pool(name="pskv", bufs=1, space="PSUM"))
            ps_q = ep(tc.tile_pool(name="psq", bufs=2, space="PSUM"))
            ps_pt = ep(tc.tile_pool(name="pspt", bufs=2, space="PSUM"))
            ps_ot = ep(tc.tile_pool(name="psot", bufs=1, space="PSUM"))
            ps_o = ep(tc.tile_pool(name="pso", bufs=2, space="PSUM"))

            # ---- resident constants / weights (order = DMA issue order) ----
            wk_s = consts.tile([128, KT, HD], bf)
            wv_s = consts.tile([128, KT, HD], bf)
            wq_s = consts.tile([128, KT, DQ], bf)
            wo_s = consts.tile([128, HPC, HIDDEN], bf)
            cosT = consts.tile([HD, S], f32)
            sinS = consts.tile([HD, S], f32)
            masks = consts.tile([128, 896], bf)
            ident = consts.tile([128, 128], bf)
            xt_s = main.tile([128, KT, 512], bf)
            kt = main.tile([128, S], bf)
            v_sb = main.tile([128, NSK, 128], bf)
            qt = main.tile([128, HPC, 512], bf)
            at = main.tile([128, HPC, S], bf)

            wk_r = wk_d.rearrange("(kt p) m -> p kt m", p=128)
            wv_r = wv_d.rearrange("(kt p) m -> p kt m", p=128)
            wq_r = wq_d.rearrange("(kt p) m -> p kt m", p=128)
            wo_r = wo_d.rearrange("(hh p) m -> p hh m", p=128)

            # K weights + first xt block first so the K pass starts ASAP
            for q in range(4):
                ksl = slice(q * 8, (q + 1) * 8)
                nc.sync.dma_start(out=wk_s[:, ksl, :], in_=wk_r[:, ksl, :])
            for k in range(KT):
                nc.sync.dma_start(out=xt_s[:, k, :],
                                  in_=xt_d[k * 128:(k + 1) * 128, 0:512])
            nc.sync.dma_start(out=cosT, in_=cos_d)
            nc.sync.dma_start(out=sinS, in_=sin_d)
            for q in range(4):
                ksl = slice(q * 8, (q + 1) * 8)
                nc.sync.dma_start(out=wq_s[:, ksl, :], in_=wq_r[:, ksl, :])
                nc.sync.dma_start(out=wv_s[:, ksl, :], in_=wv_r[:, ksl, :])
            nc.sync.dma_start(out=masks, in_=msk_d)
            nc.sync.dma_start(out=ident, in_=idn_d)
            for q in range(4):
                nsl = slice(q * 1024, (q + 1) * 1024)
                nc.sync.dma_start(out=wo_s[:, :, nsl], in_=wo_r[:, :, nsl])

            # ---- out-projection filler machinery ----
            pend = []
            done = [0]

            def filler_group():
                if not pend:
                    return
                m, n = pend.pop(0)
                o_ps = ps_o.tile([128, 512], f32, tag="ops", name="ops")
                for hh in range(HPC):
                    nc.tensor.matmul(o_ps[:],
                                     at[:, hh, m * 128:(m + 1) * 128],
                                     wo_s[:, hh, n * 512:(n + 1) * 512],
                                     start=(hh == 0), stop=(hh == HPC - 1))
                osb = osb_pool.tile([128, 512], bf, tag="osb", name="osb")
                nc.scalar.activation(osb[:], o_ps[:], CPY)
                nc.sync.dma_start(
                    out=out_d[m * 128:(m + 1) * 128, n * 512:(n + 1) * 512],
                    in_=osb[:])
                done[0] += 1

            # ---- per-block building blocks ----
            def rope(ps, dst, sl):
                """dst = rope(ps) (bf16 out) using cosT/sinS tables."""
                t = tmp_pool.tile([128, 512], f32, tag="ropet", name="ropet")
                nc.vector.tensor_mul(t[0:64, :], ps[64:128, :], sinS[0:64, sl])
                nc.vector.tensor_mul(t[64:128, :], ps[0:64, :], sinS[64:128, sl])
                nc.vector.tensor_mul(dst, ps[:], cosT[:, sl])
                nc.vector.tensor_add(dst, dst, t[:])

            def kpass():
                k_ps = ps_kv.tile([128, 512], f32, tag="kv", name="kps")
                for k in range(KT):
                    nc.tensor.matmul(k_ps[:], wk_s[:, k, :], xt_s[:, k, :],
                                     start=(k == 0), stop=(k == KT - 1))
                return k_ps

            def qpass(h):
                q_ps = ps_q.tile([128, 512], f32, tag="qps", name="qps")
                for k in range(KT):
                    nc.tensor.matmul(q_ps[:], wq_s[:, k, h * 128:(h + 1) * 128],
                                     xt_s[:, k, :],
                                     start=(k == 0), stop=(k == KT - 1))
                return q_ps

            def vpass(blk):
                v_ps = ps_kv.tile([128, 512], f32, tag="kv", name="vps")
                for k in range(KT):
                    nc.tensor.matmul(v_ps[:], wv_s[:, k, :], xt_s[:, k, :],
                                     start=(k == 0), stop=(k == KT - 1))
                vt_sb = vt_pool.tile([128, 512], bf, tag="vt", name="vt")
                nc.scalar.activation(vt_sb[:], v_ps[:], CPY)
                vtr = ps_kv.tile([128, 4, 128], bf, tag="kv", name="vtr")
                for t in range(4):
                    nc.tensor.transpose(vtr[:, t, :],
                                        vt_sb[:, t * 128:(t + 1) * 128],
                                        ident[:])
                nc.scalar.activation(v_sb[:, blk * 4:(blk + 1) * 4, :], vtr[:],
                                     CPY)

            def attn_head(h, blk, use_filler):
                lo = blk * 512
                nsk = 4 * (blk + 1)
                ot_ps = ps_ot.tile([128, 512], f32, tag="ot", name="otps")
                sums = sums_pool.tile([128, 512], f32, tag="sums", name="sums")
                for i in range(nsk):
                    pt = ps_pt.tile([128, 512], f32, tag="pt", name="pt")
                    nc.tensor.matmul(pt[:], kt[:, i * 128:(i + 1) * 128],
                                     qt[:, h, :], start=True, stop=True)
                    et = et_pool.tile([128, 512], bf, tag="et", name="et")
                    nc.scalar.activation(et[:], pt[:], EXP, scale=SCALE)
                    off = i - 4 * blk
                    if off >= 0:
                        w0 = (3 - off) * 128
                        nc.vector.tensor_mul(et[:], et[:], masks[:, w0:w0 + 512])
                    if i == 0:
                        nc.vector.tensor_copy(sums[:], et[:])
                    else:
                        nc.vector.tensor_add(sums[:], sums[:], et[:])
                    nc.tensor.matmul(ot_ps[:], v_sb[:, i, :], et[:],
                                     start=(i == 0), stop=(i == nsk - 1))
                    if use_filler and i % 2 == 1:
                        filler_group()
                sbc = sbc_pool.tile([128, 512], f32, tag="sbc", name="sbc")
                nc.gpsimd.partition_all_reduce(sbc[:], sums[:], channels=128,
                                               reduce_op=ADD)
                rec = sbc_pool.tile([128, 512], f32, tag="rec", name="rec")
                nc.vector.reciprocal_approx_fast(rec[:], sbc[:])
                nc.vector.tensor_mul(at[:, h, lo:lo + 512], ot_ps[:], rec[:])

            # -------- schedule: Q passes and out-proj interleave attention ----
            for blk in range(NBLK):
                lo = blk * 512
                sl = slice(lo, lo + 512)
                k_ps = kpass()
                rope(k_ps, kt[:, sl], sl)
                q_ps = qpass(0)
                rope(q_ps, qt[:, 0, :], sl)
                vpass(blk)
                attn_head(0, blk, blk > 0)
                for h in range(1, HPC):
                    q_ps = qpass(h)
                    rope(q_ps, qt[:, h, :], sl)
                    # drain filler toward 8*h groups before the next head
                    while pend and done[0] < 8 * h:
                        filler_group()
                    attn_head(h, blk, blk > 0)
                while pend:
                    filler_group()
                done[0] = 0
                # prefetch next block's xt + enqueue this block's out-proj
                if blk + 1 < NBLK:
                    nlo = lo + 512
                    for k in range(KT):
                        nc.sync.dma_start(
                            out=xt_s[:, k, :],
                            in_=xt_d[k * 128:(k + 1) * 128, nlo:nlo + 512])
                for m in range(4 * blk, 4 * blk + 4):
                    for n in range(HIDDEN // 512):
                        pend.append((m, n))

            # tail: last block's output projection
            while pend:
                filler_group()

    nc.compile()
    return nc


def _host_prep(hidden_states, position_ids, Wq, Wk, Wv, Wo):
    import ml_dtypes
    bf = ml_dtypes.bfloat16

    X = np.asarray(hidden_states, dtype=np.float32).reshape(S, HIDDEN)
    XT = np.ascontiguousarray(X.T).astype(bf)

    pos = np.asarray(position_ids).reshape(-1)[:S].astype(np.float32)
    inv = (1.0 / (ROPE_BASE ** (np.arange(0, HD, 2, dtype=np.float32) / HD))
           ).astype(np.float32)
    freqs = pos[:, None] * inv[None, :]              # [S, 64]
    cos_h = np.cos(freqs).astype(np.float32)         # [S, 64] (= both halves)
    sin_h = np.sin(freqs).astype(np.float32)
    cosT = np.ascontiguousarray(np.concatenate([cos_h, cos_h], axis=1).T)
    sinT = np.concatenate([sin_h, sin_h], axis=1).T
    sinS = np.ascontiguousarray(
        np.concatenate([-sinT[0:64], sinT[64:128]], axis=0))

    # sliding-window mask [zeros(3x128) | tri | ones(3x128)]: offset o slice
    # starts at (3-o)*128 and covers 512 cols -> c<o zero, c==o tri, c>o ones
    tri = (np.arange(128)[:, None] <= np.arange(128)[None, :]).astype(bf)
    masks = np.concatenate([np.zeros((128, 384), bf), tri,
                            np.ones((128, 384), bf)], axis=1)

    ident = np.eye(128, dtype=bf)

    Wq = np.asarray(Wq, dtype=np.float32)
    Wk = np.asarray(Wk, dtype=np.float32)
    Wv = np.asarray(Wv, dtype=np.float32)
    Wo = np.asarray(Wo, dtype=np.float32)

    in_maps = []
    for c in range(N_CORES):
        in_maps.append({
            "xt": XT,
            "wq": np.ascontiguousarray(Wq[:, c * DQ:(c + 1) * DQ]).astype(bf),
            "wk": np.ascontiguousarray(Wk[:, c * HD:(c + 1) * HD]).astype(bf),
            "wv": np.ascontiguousarray(Wv[:, c * HD:(c + 1) * HD]).astype(bf),
            "wo": np.ascontiguousarray(Wo[c * DQ:(c + 1) * DQ, :]).astype(bf),
            "cosT": cosT,
            "sinS": sinS,
            "masks": masks,
            "ident": ident,
        })
    return in_maps


def kernel(hidden_states, position_ids, Wq, Wk, Wv, Wo, _run_opts=None):
    from concourse.bass_utils import run_bass_kernel_spmd

    if "nc" not in _CACHE:
        _CACHE["nc"] = _build()
    nc = _CACHE["nc"]

    in_maps = _host_prep(hidden_states, position_ids, Wq, Wk, Wv, Wo)
    opts = dict(_run_opts or {})
    res = run_bass_kernel_spmd(nc, in_maps, core_ids=list(range(N_CORES)), **opts)
    _CACHE["last_result"] = res

    out = res.results[0]["out"].astype(np.float64)
    for c in range(1, N_CORES):
        out += res.results[c]["out"].astype(np.float64)
    return out.astype(np.float32).reshape(1, S, HIDDEN)


# revision 8
# speedup vs baseline: 1.9608x; 1.0313x over previous
"""LlamaAttention (B=1, S=2048, H=4096, 32 heads / 8 KV heads) on 8 TRN2 NeuronCores.

Sharding: tensor-parallel over heads. Core c owns Q heads [4c, 4c+4) and KV head c
(Wq/Wk/Wv column shards, Wo row shard). Each core computes a full [S, H] partial
output in bf16; the host sums the 8 partials (the all-reduce for row-sharded Wo).

v2 vs baseline: all matmul operands bf16 (half DMA/SBUF traffic, same PE rate),
all weights SBUF-resident (Wq was re-streamed 4x), xt block-resident with
prefetch, per-head projection passes to cut concurrent PSUM banks 6->4, PSUM
rebalanced so attention double-buffers PT, softmax tail shortened
(reciprocal_approx_fast instead of 3.3us DVE reciprocal) and hidden behind
out-projection filler matmuls interleaved into the next head's attention loop
(keeps the PE warm - the baseline re-throttled to 1.2 GHz at every head tail).
"""

import numpy as np

HIDDEN = 4096
N_HEADS = 32
N_KV = 8
HD = 128
S = 2048
N_CORES = 8
HPC = N_HEADS // N_CORES          # 4 Q heads per core
DQ = HPC * HD                     # 512 q columns per core
ROPE_BASE = 10000.0
SCALE = 1.0 / float(np.sqrt(HD))

NBLK = S // 512                   # 4 sq blocks of 512
NSK = S // 128                    # 16 sk tiles of 128
KT = HIDDEN // 128                # 32 contraction tiles

_CACHE = {}


def _build():
    import concourse.bass as bass
    import concourse.tile as tile
    from concourse import bacc, mybir
    import concourse.bass_isa as bass_isa

    f32 = mybir.dt.float32
    bf = mybir.dt.bfloat16
    EXP = mybir.ActivationFunctionType.Exp
    CPY = mybir.ActivationFunctionType.Copy
    ADD = bass_isa.ReduceOp.add

    nc = bacc.Bacc("TRN2", target_bir_lowering=False, debug=False,
                   num_devices=N_CORES)

    xt_d = nc.dram_tensor("xt", [HIDDEN, S], bf, kind="ExternalInput").ap()
    wq_d = nc.dram_tensor("wq", [HIDDEN, DQ], bf, kind="ExternalInput").ap()
    wk_d = nc.dram_tensor("wk", [HIDDEN, HD], bf, kind="ExternalInput").ap()
    wv_d = nc.dram_tensor("wv", [HIDDEN, HD], bf, kind="ExternalInput").ap()
    wo_d = nc.dram_tensor("wo", [DQ, HIDDEN], bf, kind="ExternalInput").ap()
    cos_d = nc.dram_tensor("cosT", [HD, S], f32, kind="ExternalInput").ap()
    sin_d = nc.dram_tensor("sinS", [HD, S], f32, kind="ExternalInput").ap()
    msk_d = nc.dram_tensor("masks", [128, 896], bf, kind="ExternalInput").ap()
    idn_d = nc.dram_tensor("ident", [128, 128], bf, kind="ExternalInput").ap()
    out_d = nc.dram_tensor("out", [S, HIDDEN], bf, kind="ExternalOutput").ap()

    with tile.TileContext(nc) as tc:
        from contextlib import ExitStack
        with ExitStack() as ctx:
            ep = ctx.enter_context
            consts = ep(tc.tile_pool(name="consts", bufs=1))
            main = ep(tc.tile_pool(name="main", bufs=1))
            tmp_pool = ep(tc.tile_pool(name="tmpp", bufs=2))
            et_pool = ep(tc.tile_pool(name="etp", bufs=8))
            sums_pool = ep(tc.tile_pool(name="sumsp", bufs=2))
            sbc_pool = ep(tc.tile_pool(name="sbcp", bufs=2))
            vt_pool = ep(tc.tile_pool(name="vtp", bufs=2))
            osb_pool = ep(tc.tile_pool(name="osbp", bufs=3))
            ps_kv = ep(tc.tile_pool(name="pskv", bufs=1, space="PSUM"))
            ps_q = ep(tc.tile_pool(name="psq", bufs=1, space="PSUM"))
            ps_pt = ep(tc.tile_pool(name="pspt", bufs=2, space="PSUM"))
            ps_ot = ep(tc.tile_pool(name="psot", bufs=2, space="PSUM"))
            ps_o = ep(tc.tile_pool(name="pso", bufs=2, space="PSUM"))

            # ---- resident constants / weights (order = DMA issue order) ----
            wk_s = consts.tile([128, KT, HD], bf)
            wv_s = consts.tile([128, KT, HD], bf)
            wq_s = consts.tile([128, KT, DQ], bf)
            wo_s = consts.tile([128, HPC, HIDDEN], bf)
            cosT = consts.tile([HD, S], f32)
            sinS = consts.tile([HD, S], f32)
            masks = consts.tile([128, 896], bf)
            ident = consts.tile([128, 128], bf)
            xt_s = main.tile([128, KT, 512], bf)
            kt = main.tile([128, S], bf)
            v_sb = main.tile([128, NSK, 128], bf)
            qt = main.tile([128, HPC, 512], bf)
            at = main.tile([128, HPC, S], bf)

            wk_r = wk_d.rearrange("(kt p) m -> p kt m", p=128)
            wv_r = wv_d.rearrange("(kt p) m -> p kt m", p=128)
            wq_r = wq_d.rearrange("(kt p) m -> p kt m", p=128)
            wo_r = wo_d.rearrange("(hh p) m -> p hh m", p=128)

            # K weights + first xt block first so the K pass starts ASAP;
            # wq/wv next (Q0/V pass dependencies), rope tables and Wo later.
            xt_r = xt_d.rearrange("(kt p) s -> p kt s", p=128)
            for q in range(4):
                ksl = slice(q * 8, (q + 1) * 8)
                nc.sync.dma_start(out=wk_s[:, ksl, :], in_=wk_r[:, ksl, :])
            for q in range(8):
                ksl = slice(q * 4, (q + 1) * 4)
                nc.sync.dma_start(out=xt_s[:, ksl, :],
                                  in_=xt_r[:, ksl, 0:512])
            for q in range(4):
                ksl = slice(q * 8, (q + 1) * 8)
                nc.sync.dma_start(out=wq_s[:, ksl, :], in_=wq_r[:, ksl, :])
            for q in range(4):
                ksl = slice(q * 8, (q + 1) * 8)
                nc.sync.dma_start(out=wv_s[:, ksl, :], in_=wv_r[:, ksl, :])
            nc.sync.dma_start(out=cosT, in_=cos_d)
            nc.sync.dma_start(out=sinS, in_=sin_d)
            nc.sync.dma_start(out=masks, in_=msk_d)
            nc.sync.dma_start(out=ident, in_=idn_d)
            for q in range(4):
                nsl = slice(q * 1024, (q + 1) * 1024)
                nc.sync.dma_start(out=wo_s[:, :, nsl], in_=wo_r[:, :, nsl])

            # PE warm-up: dummy matmuls on an un-initialized tile get the HAM
            # clock gate to 8/8 before the first real matmul arrives.
            wrm = main.tile([128, 512], bf)
            nc.vector.memset(wrm[:], 0.0)
            for w in range(16):
                wps = ps_pt.tile([128, 512], f32, tag="pt", name="wps")
                nc.tensor.matmul(wps[:], wrm[:, 0:128], wrm[:],
                                 start=True, stop=True)

            # ---- out-projection filler machinery (n-pair granularity) ----
            pend = []
            done = [0]

            def filler_group():
                """One filler unit: out rows m*128..+128, cols np2*1024..+1024
                (8 matmuls -> 2 PSUM groups -> one [128,1024] bf16 store)."""
                if not pend:
                    return
                m, np2 = pend.pop(0)
                osb = osb_pool.tile([128, 1024], bf, tag="osb", name="osb")
                for half in range(2):
                    n = np2 * 2 + half
                    o_ps = ps_o.tile([128, 512], f32, tag="ops", name="ops")
                    for hh in range(HPC):
                        nc.tensor.matmul(o_ps[:],
                                         at[:, hh, m * 128:(m + 1) * 128],
                                         wo_s[:, hh, n * 512:(n + 1) * 512],
                                         start=(hh == 0), stop=(hh == HPC - 1))
                    nc.scalar.activation(osb[:, half * 512:(half + 1) * 512],
                                         o_ps[:], CPY)
                nc.sync.dma_start(
                    out=out_d[m * 128:(m + 1) * 128,
                              np2 * 1024:(np2 + 1) * 1024],
                    in_=osb[:])
                done[0] += 1

            # ---- per-block building blocks ----
            def rope(ps, dst, sl):
                """dst = rope(ps) (bf16 out) using cosT/sinS tables."""
                t = tmp_pool.tile([128, 512], f32, tag="ropet", name="ropet")
                nc.vector.tensor_mul(t[0:64, :], ps[64:128, :], sinS[0:64, sl])
                nc.vector.tensor_mul(t[64:128, :], ps[0:64, :], sinS[64:128, sl])
                nc.vector.tensor_mul(dst, ps[:], cosT[:, sl])
                nc.vector.tensor_add(dst, dst, t[:])

            def kpass():
                k_ps = ps_kv.tile([128, 512], f32, tag="kv", name="kps")
                for k in range(KT):
                    nc.tensor.matmul(k_ps[:], wk_s[:, k, :], xt_s[:, k, :],
                                     start=(k == 0), stop=(k == KT - 1))
                return k_ps

            def qpass(h):
                q_ps = ps_q.tile([128, 512], f32, tag="qps", name="qps")
                for k in range(KT):
                    nc.tensor.matmul(q_ps[:], wq_s[:, k, h * 128:(h + 1) * 128],
                                     xt_s[:, k, :],
                                     start=(k == 0), stop=(k == KT - 1))
                return q_ps

            def vpass(blk):
                v_ps = ps_kv.tile([128, 512], f32, tag="kv", name="vps")
                for k in range(KT):
                    nc.tensor.matmul(v_ps[:], wv_s[:, k, :], xt_s[:, k, :],
                                     start=(k == 0), stop=(k == KT - 1))
                vt_sb = vt_pool.tile([128, 512], bf, tag="vt", name="vt")
                nc.scalar.activation(vt_sb[:], v_ps[:], CPY)
                vtr = ps_kv.tile([128, 4, 128], bf, tag="kv", name="vtr")
                for t in range(4):
                    nc.tensor.transpose(vtr[:, t, :],
                                        vt_sb[:, t * 128:(t + 1) * 128],
                                        ident[:])
                nc.scalar.activation(v_sb[:, blk * 4:(blk + 1) * 4, :], vtr[:],
                                     CPY)

            def attn_head(h, blk, use_filler):
                lo = blk * 512
                nsk = 4 * (blk + 1)
                ot_ps = ps_ot.tile([128, 512], f32, tag="ot", name="otps")
                sums = sums_pool.tile([128, 512], f32, tag="sums", name="sums")
                for i in range(nsk):
                    pt = ps_pt.tile([128, 512], f32, tag="pt", name="pt")
                    nc.tensor.matmul(pt[:], kt[:, i * 128:(i + 1) * 128],
                                     qt[:, h, :], start=True, stop=True)
                    et = et_pool.tile([128, 512], bf, tag="et", name="et")
                    nc.scalar.activation(et[:], pt[:], EXP, scale=SCALE)
                    off = i - 4 * blk
                    if off >= 0:
                        w0 = (3 - off) * 128
                        nc.vector.tensor_mul(et[:], et[:], masks[:, w0:w0 + 512])
                    if i == 0:
                        nc.vector.tensor_copy(sums[:], et[:])
                    else:
                        nc.vector.tensor_add(sums[:], sums[:], et[:])
                    nc.tensor.matmul(ot_ps[:], v_sb[:, i, :], et[:],
                                     start=(i == 0), stop=(i == nsk - 1))
                    if use_filler and i % 3 == 2:
                        filler_group()
                sbc = sbc_pool.tile([128, 512], f32, tag="sbc", name="sbc")
                nc.gpsimd.partition_all_reduce(sbc[:], sums[:], channels=128,
                                               reduce_op=ADD)
                rec = sbc_pool.tile([128, 512], f32, tag="rec", name="rec")
                nc.vector.reciprocal_approx_fast(rec[:], sbc[:])
                nc.vector.tensor_mul(at[:, h, lo:lo + 512], ot_ps[:], rec[:])

            # -------- schedule: Q passes and out-proj interleave attention ----
            for blk in range(NBLK):
                lo = blk * 512
                sl = slice(lo, lo + 512)
                k_ps = kpass()
                rope(k_ps, kt[:, sl], sl)
                q_ps = qpass(0)
                rope(q_ps, qt[:, 0, :], sl)
                vpass(blk)
                attn_head(0, blk, blk > 0)
                for h in range(1, HPC):
                    q_ps = qpass(h)
                    rope(q_ps, qt[:, h, :], sl)
                    # drain filler toward 4*h pair-groups before the next head
                    while pend and done[0] < 4 * h:
                        filler_group()
                    attn_head(h, blk, blk > 0)
                # drain; on the last block keep 3 pair-groups to cover the
                # final head's softmax tail
                keep = 3 if blk == NBLK - 1 else 0
                while len(pend) > keep:
                    filler_group()
                done[0] = 0
                # prefetch next block's xt + enqueue this block's out-proj
                if blk + 1 < NBLK:
                    nlo = lo + 512
                    for q in range(8):
                        ksl = slice(q * 4, (q + 1) * 4)
                        nc.sync.dma_start(out=xt_s[:, ksl, :],
                                          in_=xt_r[:, ksl, nlo:nlo + 512])
                for m in range(4 * blk, 4 * blk + 4):
                    for np2 in range(HIDDEN // 1024):
                        pend.append((m, np2))

            # tail: last block's output projection
            while pend:
                filler_group()

    nc.compile()
    return nc


def _host_prep(hidden_states, position_ids, Wq, Wk, Wv, Wo):
    import ml_dtypes
    bf = ml_dtypes.bfloat16

    X = np.asarray(hidden_states, dtype=np.float32).reshape(S, HIDDEN)
    XT = np.ascontiguousarray(X.T).astype(bf)

    pos = np.asarray(position_ids).reshape(-1)[:S].astype(np.float32)
    inv = (1.0 / (ROPE_BASE ** (np.arange(0, HD, 2, dtype=np.float32) / HD))
           ).astype(np.float32)
    freqs = pos[:, None] * inv[None, :]              # [S, 64]
    cos_h = np.cos(freqs).astype(np.float32)         # [S, 64] (= both halves)
    sin_h = np.sin(freqs).astype(np.float32)
    cosT = np.ascontiguousarray(np.concatenate([cos_h, cos_h], axis=1).T)
    sinT = np.concatenate([sin_h, sin_h], axis=1).T
    sinS = np.ascontiguousarray(
        np.concatenate([-sinT[0:64], sinT[64:128]], axis=0))

    # sliding-window mask [zeros(3x128) | tri | ones(3x128)]: offset o slice
    # starts at (3-o)*128 and covers 512 cols -> c<o zero, c==o tri, c>o ones
    tri = (np.arange(128)[:, None] <= np.arange(128)[None, :]).astype(bf)
    masks = np.concatenate([np.zeros((128, 384), bf), tri,
                            np.ones((128, 384), bf)], axis=1)

    ident = np.eye(128, dtype=bf)

    Wq = np.asarray(Wq, dtype=np.float32)
    Wk = np.asarray(Wk, dtype=np.float32)
    Wv = np.asarray(Wv, dtype=np.float32)
    Wo = np.asarray(Wo, dtype=np.float32)

    in_maps = []
    for c in range(N_CORES):
        in_maps.append({
            "xt": XT,
            "wq": np.ascontiguousarray(Wq[:, c * DQ:(c + 1) * DQ]).astype(bf),
            "wk": np.ascontiguousarray(Wk[:, c * HD:(c + 1) * HD]).astype(bf),
            "wv": np.ascontiguousarray(Wv[:, c * HD:(c + 1) * HD]).astype(bf),
            "wo": np.ascontiguousarray(Wo[c * DQ:(c + 1) * DQ, :]).astype(bf),
            "cosT": cosT,
            "sinS": sinS,
            "masks": masks,
            "ident": ident,
        })
    return in_maps


def kernel(hidden_states, position_ids, Wq, Wk, Wv, Wo, _run_opts=None):
    from concourse.bass_utils import run_bass_kernel_spmd

    if "nc" not in _CACHE:
        _CACHE["nc"] = _build()
    nc = _CACHE["nc"]

    in_maps = _host_prep(hidden_states, position_ids, Wq, Wk, Wv, Wo)
    opts = dict(_run_opts or {})
    res = run_bass_kernel_spmd(nc, in_maps, core_ids=list(range(N_CORES)), **opts)
    _CACHE["last_result"] = res

    out = res.results[0]["out"].astype(np.float64)
    for c in range(1, N_CORES):
        out += res.results[c]["out"].astype(np.float64)
    return out.astype(np.float32).reshape(1, S, HIDDEN)


# revision 11
# speedup vs baseline: 2.1026x; 1.0723x over previous
"""LlamaAttention (B=1, S=2048, H=4096, 32 heads / 8 KV heads) on 8 TRN2 NeuronCores.

Sharding: tensor-parallel over heads. Core c owns Q heads [4c, 4c+4) and KV head c
(Wq/Wk/Wv column shards, Wo row shard). Each core computes a full [S, H] partial
output in bf16; the host sums the 8 partials (the all-reduce for row-sharded Wo).

v2 vs baseline: all matmul operands bf16 (half DMA/SBUF traffic, same PE rate),
all weights SBUF-resident (Wq was re-streamed 4x), xt block-resident with
prefetch, per-head projection passes to cut concurrent PSUM banks 6->4, PSUM
rebalanced so attention double-buffers PT, softmax tail shortened
(reciprocal_approx_fast instead of 3.3us DVE reciprocal) and hidden behind
out-projection filler matmuls interleaved into the next head's attention loop
(keeps the PE warm - the baseline re-throttled to 1.2 GHz at every head tail).
"""

import numpy as np

HIDDEN = 4096
N_HEADS = 32
N_KV = 8
HD = 128
S = 2048
N_CORES = 8
HPC = N_HEADS // N_CORES          # 4 Q heads per core
DQ = HPC * HD                     # 512 q columns per core
ROPE_BASE = 10000.0
SCALE = 1.0 / float(np.sqrt(HD))

NBLK = S // 512                   # 4 sq blocks of 512
NSK = S // 128                    # 16 sk tiles of 128
KT = HIDDEN // 128                # 32 contraction tiles

_CACHE = {}


def _build():
    import concourse.bass as bass
    import concourse.tile as tile
    from concourse import bacc, mybir
    import concourse.bass_isa as bass_isa

    f32 = mybir.dt.float32
    bf = mybir.dt.bfloat16
    EXP = mybir.ActivationFunctionType.Exp
    CPY = mybir.ActivationFunctionType.Copy
    ADD = bass_isa.ReduceOp.add

    nc = bacc.Bacc("TRN2", target_bir_lowering=False, debug=False,
                   num_devices=N_CORES)

    xt_d = nc.dram_tensor("xt", [HIDDEN, S], bf, kind="ExternalInput").ap()
    wq_d = nc.dram_tensor("wq", [HIDDEN, DQ], bf, kind="ExternalInput").ap()
    wk_d = nc.dram_tensor("wk", [HIDDEN, HD], bf, kind="ExternalInput").ap()
    wv_d = nc.dram_tensor("wv", [HIDDEN, HD], bf, kind="ExternalInput").ap()
    wo_d = nc.dram_tensor("wo", [DQ, HIDDEN], bf, kind="ExternalInput").ap()
    cos_d = nc.dram_tensor("cosT", [HD, S], f32, kind="ExternalInput").ap()
    sin_d = nc.dram_tensor("sinS", [HD, S], f32, kind="ExternalInput").ap()
    msk_d = nc.dram_tensor("masks", [128, 896], bf, kind="ExternalInput").ap()
    idn_d = nc.dram_tensor("ident", [128, 128], bf, kind="ExternalInput").ap()
    out_d = nc.dram_tensor("out", [S, HIDDEN], bf, kind="ExternalOutput").ap()

    with tile.TileContext(nc) as tc:
        from contextlib import ExitStack
        with ExitStack() as ctx:
            ep = ctx.enter_context
            consts = ep(tc.tile_pool(name="consts", bufs=1))
            main = ep(tc.tile_pool(name="main", bufs=1))
            tmp_pool = ep(tc.tile_pool(name="tmpp", bufs=2))
            et_pool = ep(tc.tile_pool(name="etp", bufs=8))
            sums_pool = ep(tc.tile_pool(name="sumsp", bufs=2))
            sbc_pool = ep(tc.tile_pool(name="sbcp", bufs=2))
            vt_pool = ep(tc.tile_pool(name="vtp", bufs=2))
            osb_pool = ep(tc.tile_pool(name="osbp", bufs=3))
            ps_kv = ep(tc.tile_pool(name="pskv", bufs=1, space="PSUM"))
            ps_q = ep(tc.tile_pool(name="psq", bufs=1, space="PSUM"))
            ps_pt = ep(tc.tile_pool(name="pspt", bufs=2, space="PSUM"))
            ps_ot = ep(tc.tile_pool(name="psot", bufs=2, space="PSUM"))
            ps_o = ep(tc.tile_pool(name="pso", bufs=2, space="PSUM"))

            # ---- resident constants / weights (order = DMA issue order) ----
            wk_s = consts.tile([128, KT, HD], bf)
            wv_s = consts.tile([128, KT, HD], bf)
            wq_s = consts.tile([128, KT, DQ], bf)
            wo_s = consts.tile([128, HPC, HIDDEN], bf)
            cosT = consts.tile([HD, S], f32)
            sinS = consts.tile([HD, S], f32)
            masks = consts.tile([128, 896], bf)
            ident = consts.tile([128, 128], bf)
            xt_s = main.tile([128, KT, 512], bf)
            kt = main.tile([128, S], bf)
            v_sb = main.tile([128, NSK, 128], bf)
            qt = main.tile([128, HPC, 512], bf)
            at = main.tile([128, HPC, S], bf)

            wk_r = wk_d.rearrange("(kt p) m -> p kt m", p=128)
            wv_r = wv_d.rearrange("(kt p) m -> p kt m", p=128)
            wq_r = wq_d.rearrange("(kt p) m -> p kt m", p=128)
            wo_r = wo_d.rearrange("(hh p) m -> p hh m", p=128)

            # K weights + first xt block first so the K pass starts ASAP;
            # wq/wv next (Q0/V pass dependencies), rope tables and Wo later.
            xt_r = xt_d.rearrange("(kt p) s -> p kt s", p=128)
            for q in range(4):
                ksl = slice(q * 8, (q + 1) * 8)
                nc.sync.dma_start(out=wk_s[:, ksl, :], in_=wk_r[:, ksl, :])
            for q in range(8):
                ksl = slice(q * 4, (q + 1) * 4)
                nc.sync.dma_start(out=xt_s[:, ksl, :],
                                  in_=xt_r[:, ksl, 0:512])
            # rope tables + mask before the big weights: rope-K gates attn h0
            nc.sync.dma_start(out=cosT, in_=cos_d)
            nc.sync.dma_start(out=sinS, in_=sin_d)
            nc.sync.dma_start(out=masks, in_=msk_d)
            nc.sync.dma_start(out=ident, in_=idn_d)
            for q in range(4):
                ksl = slice(q * 8, (q + 1) * 8)
                nc.sync.dma_start(out=wq_s[:, ksl, :], in_=wq_r[:, ksl, :])
            for q in range(4):
                ksl = slice(q * 8, (q + 1) * 8)
                nc.sync.dma_start(out=wv_s[:, ksl, :], in_=wv_r[:, ksl, :])
            for q in range(4):
                nsl = slice(q * 1024, (q + 1) * 1024)
                nc.sync.dma_start(out=wo_s[:, :, nsl], in_=wo_r[:, :, nsl])

            # PE warm-up: dummy matmuls on an un-initialized tile get the HAM
            # clock gate to 8/8 before the first real matmul arrives.
            wrm = main.tile([128, 512], bf)
            nc.vector.memset(wrm[:], 0.0)
            for w in range(16):
                wps = ps_pt.tile([128, 512], f32, tag="pt", name="wps")
                nc.tensor.matmul(wps[:], wrm[:, 0:128], wrm[:],
                                 start=True, stop=True)

            # ---- out-projection filler machinery (n-pair granularity) ----
            pend = []
            done = [0]

            def filler_group():
                """One filler unit: out rows m*128..+128, cols np2*1024..+1024
                (8 matmuls -> 2 PSUM groups -> one [128,1024] bf16 store)."""
                if not pend:
                    return
                m, np2 = pend.pop(0)
                osb = osb_pool.tile([128, 1024], bf, tag="osb", name="osb")
                for half in range(2):
                    n = np2 * 2 + half
                    o_ps = ps_o.tile([128, 512], f32, tag="ops", name="ops")
                    for hh in range(HPC):
                        nc.tensor.matmul(o_ps[:],
                                         at[:, hh, m * 128:(m + 1) * 128],
                                         wo_s[:, hh, n * 512:(n + 1) * 512],
                                         start=(hh == 0), stop=(hh == HPC - 1))
                    nc.scalar.activation(osb[:, half * 512:(half + 1) * 512],
                                         o_ps[:], CPY)
                nc.sync.dma_start(
                    out=out_d[m * 128:(m + 1) * 128,
                              np2 * 1024:(np2 + 1) * 1024],
                    in_=osb[:])
                done[0] += 1

            # ---- per-block building blocks ----
            def rope(ps, dst, sl):
                """dst = rope(ps) (bf16 out) using cosT/sinS tables."""
                t = tmp_pool.tile([128, 512], f32, tag="ropet", name="ropet")
                nc.vector.tensor_mul(t[0:64, :], ps[64:128, :], sinS[0:64, sl])
                nc.vector.tensor_mul(t[64:128, :], ps[0:64, :], sinS[64:128, sl])
                nc.vector.tensor_mul(dst, ps[:], cosT[:, sl])
                nc.vector.tensor_add(dst, dst, t[:])

            def kpass():
                k_ps = ps_kv.tile([128, 512], f32, tag="kv", name="kps")
                for k in range(KT):
                    nc.tensor.matmul(k_ps[:], wk_s[:, k, :], xt_s[:, k, :],
                                     start=(k == 0), stop=(k == KT - 1))
                return k_ps

            def qpass(h):
                q_ps = ps_q.tile([128, 512], f32, tag="qps", name="qps")
                for k in range(KT):
                    nc.tensor.matmul(q_ps[:], wq_s[:, k, h * 128:(h + 1) * 128],
                                     xt_s[:, k, :],
                                     start=(k == 0), stop=(k == KT - 1))
                return q_ps

            def vpass(blk):
                v_ps = ps_kv.tile([128, 512], f32, tag="kv", name="vps")
                for k in range(KT):
                    nc.tensor.matmul(v_ps[:], wv_s[:, k, :], xt_s[:, k, :],
                                     start=(k == 0), stop=(k == KT - 1))
                vt_sb = vt_pool.tile([128, 512], bf, tag="vt", name="vt")
                nc.scalar.activation(vt_sb[:], v_ps[:], CPY)
                vtr = ps_kv.tile([128, 4, 128], bf, tag="kv", name="vtr")
                for t in range(4):
                    nc.tensor.transpose(vtr[:, t, :],
                                        vt_sb[:, t * 128:(t + 1) * 128],
                                        ident[:])
                nc.scalar.activation(v_sb[:, blk * 4:(blk + 1) * 4, :], vtr[:],
                                     CPY)

            def attn_head(h, blk, use_filler):
                lo = blk * 512
                nsk = 4 * (blk + 1)
                ot_ps = ps_ot.tile([128, 512], f32, tag="ot", name="otps")
                sums = sums_pool.tile([128, 512], f32, tag="sums", name="sums")
                for i in range(nsk):
                    off = i - 4 * blk
                    # diagonal-band tiles: queries below off*128 see nothing
                    c0 = off * 128 if off > 0 else 0
                    qs = slice(c0, 512)
                    pt = ps_pt.tile([128, 512], f32, tag="pt", name="pt")
                    nc.tensor.matmul(pt[:, qs], kt[:, i * 128:(i + 1) * 128],
                                     qt[:, h, qs], start=True, stop=True)
                    et = et_pool.tile([128, 512], bf, tag="et", name="et")
                    nc.scalar.activation(et[:, qs], pt[:, qs], EXP, scale=SCALE)
                    if off >= 0:
                        nc.vector.tensor_mul(et[:, qs], et[:, qs],
                                             masks[:, 384:896 - c0])
                    if i == 0:
                        nc.vector.tensor_copy(sums[:], et[:])
                    else:
                        nc.vector.tensor_add(sums[:, qs], sums[:, qs],
                                             et[:, qs])
                    nc.tensor.matmul(ot_ps[:, qs], v_sb[:, i, :], et[:, qs],
                                     start=(i == 0), stop=(i == nsk - 1))
                    if use_filler and i % 3 == 2:
                        filler_group()
                sbc = sbc_pool.tile([128, 512], f32, tag="sbc", name="sbc")
                nc.gpsimd.partition_all_reduce(sbc[:], sums[:], channels=128,
                                               reduce_op=ADD)
                rec = sbc_pool.tile([128, 512], f32, tag="rec", name="rec")
                nc.vector.reciprocal_approx_fast(rec[:], sbc[:])
                nc.vector.tensor_mul(at[:, h, lo:lo + 512], ot_ps[:], rec[:])

            # -------- schedule --------
            # Per block: K, Q0, V, T, Q1, h0, Q2, h1, Q3, h2, h3.
            # qpass(h+1)+rope run BEFORE attn_head(h): the Q-pass matmuls are
            # the PE filler over head h's softmax tail, and rope-Q(h+1) lands
            # on the DVE queue ahead of head h's sums so qt[h+1] is ready.
            reserve = []
            for blk in range(NBLK):
                lo = blk * 512
                sl = slice(lo, lo + 512)
                if blk == NBLK - 1:
                    # hold back filler to cover the final head's softmax tail
                    for _ in range(min(5, len(pend))):
                        reserve.append(pend.pop(0))
                k_ps = kpass()
                rope(k_ps, kt[:, sl], sl)
                q_ps = qpass(0)
                rope(q_ps, qt[:, 0, :], sl)
                vpass(blk)
                q_ps = qpass(1)
                rope(q_ps, qt[:, 1, :], sl)
                for h in range(HPC):
                    attn_head(h, blk, blk > 0)
                    if h + 2 < HPC:
                        q_ps = qpass(h + 2)
                        rope(q_ps, qt[:, h + 2, :], sl)
                    while pend and done[0] < 4 * (h + 1):
                        filler_group()
                done[0] = 0
                # prefetch next block's xt + enqueue this block's out-proj
                if blk + 1 < NBLK:
                    nlo = lo + 512
                    for q in range(8):
                        ksl = slice(q * 4, (q + 1) * 4)
                        nc.sync.dma_start(out=xt_s[:, ksl, :],
                                          in_=xt_r[:, ksl, nlo:nlo + 512])
                for m in range(4 * blk, 4 * blk + 4):
                    for np2 in range(HIDDEN // 1024):
                        pend.append((m, np2))

            # tail: reserved pairs (immediately runnable, hide the last
            # softmax tail) then the last block's output projection
            pend[0:0] = reserve
            while pend:
                filler_group()

    nc.compile()
    return nc


def _host_prep(hidden_states, position_ids, Wq, Wk, Wv, Wo):
    import ml_dtypes
    bf = ml_dtypes.bfloat16

    X = np.asarray(hidden_states, dtype=np.float32).reshape(S, HIDDEN)
    XT = np.ascontiguousarray(X.T).astype(bf)

    pos = np.asarray(position_ids).reshape(-1)[:S].astype(np.float32)
    inv = (1.0 / (ROPE_BASE ** (np.arange(0, HD, 2, dtype=np.float32) / HD))
           ).astype(np.float32)
    freqs = pos[:, None] * inv[None, :]              # [S, 64]
    cos_h = np.cos(freqs).astype(np.float32)         # [S, 64] (= both halves)
    sin_h = np.sin(freqs).astype(np.float32)
    cosT = np.ascontiguousarray(np.concatenate([cos_h, cos_h], axis=1).T)
    sinT = np.concatenate([sin_h, sin_h], axis=1).T
    sinS = np.ascontiguousarray(
        np.concatenate([-sinT[0:64], sinT[64:128]], axis=0))

    # sliding-window mask [zeros(3x128) | tri | ones(3x128)]: offset o slice
    # starts at (3-o)*128 and covers 512 cols -> c<o zero, c==o tri, c>o ones
    tri = (np.arange(128)[:, None] <= np.arange(128)[None, :]).astype(bf)
    masks = np.concatenate([np.zeros((128, 384), bf), tri,
                            np.ones((128, 384), bf)], axis=1)

    ident = np.eye(128, dtype=bf)

    Wq = np.asarray(Wq, dtype=np.float32)
    Wk = np.asarray(Wk, dtype=np.float32)
    Wv = np.asarray(Wv, dtype=np.float32)
    Wo = np.asarray(Wo, dtype=np.float32)

    in_maps = []
    for c in range(N_CORES):
        in_maps.append({
            "xt": XT,
            "wq": np.ascontiguousarray(Wq[:, c * DQ:(c + 1) * DQ]).astype(bf),
            "wk": np.ascontiguousarray(Wk[:, c * HD:(c + 1) * HD]).astype(bf),
            "wv": np.ascontiguousarray(Wv[:, c * HD:(c + 1) * HD]).astype(bf),
            "wo": np.ascontiguousarray(Wo[c * DQ:(c + 1) * DQ, :]).astype(bf),
            "cosT": cosT,
            "sinS": sinS,
            "masks": masks,
            "ident": ident,
        })
    return in_maps


def kernel(hidden_states, position_ids, Wq, Wk, Wv, Wo, _run_opts=None):
    from concourse.bass_utils import run_bass_kernel_spmd

    if "nc" not in _CACHE:
        _CACHE["nc"] = _build()
    nc = _CACHE["nc"]

    in_maps = _host_prep(hidden_states, position_ids, Wq, Wk, Wv, Wo)
    opts = dict(_run_opts or {})
    res = run_bass_kernel_spmd(nc, in_maps, core_ids=list(range(N_CORES)), **opts)
    _CACHE["last_result"] = res

    out = res.results[0]["out"].astype(np.float64)
    for c in range(1, N_CORES):
        out += res.results[c]["out"].astype(np.float64)
    return out.astype(np.float32).reshape(1, S, HIDDEN)
